# revision 1
# baseline (speedup 1.0000x reference)
"""Bass/Trainium2 kernel for DropConnect (training path, Wstd != 0).

Z[b,o] = sum_i X[b,i] * W[i,o] * Werr[loc_id[b],i,o] + bias[o] * Berr[loc_id[b],o]

Strategy (8 NeuronCores, data-parallel over batch):
  - each core handles 16 samples; W/bias and the Werr/Berr pools are replicated
  - per sample, the 1MB Werr[loc] slab is gathered on-device with one indirect
    DMA: Werr viewed as [128000, 2048] macro-rows, dest partition p pulls the
    contiguous 8KB macro-row loc*128+p (i.e. input rows i=4p..4p+3)
  - VectorE multiplies the slab elementwise with W (same macro-row layout)
  - TensorE contracts with X: for j in 0..3 the column X[b, 4p+j] is the
    stationary [128,1] operand against the [128,512] slice j of the product,
    accumulating into a [1,512] PSUM tile; a 5th matmul with a one-hot [16,1]
    column against the precomputed bias*Berr[loc] [16,512] tile adds the bias
  - ScalarE copies each sample's PSUM row into a [1, 8192] staging tile which
    is stored with a single DMA
"""

import sys

sys.path.insert(0, "/opt/trn_rl_repo")

import numpy as np

B, IN, OUT, POOL, NCORES = 128, 512, 512, 1000, 8
BL = B // NCORES  # samples per core
WT_COLS = 4 * OUT  # 2048: one macro-row = 4 input rows of W/Werr

_CACHE = {}


def _build(pool_entries=POOL):
    import concourse.bass as bass
    import concourse.mybir as mybir
    import concourse.tile as tile
    from concourse import bacc

    f32, i32 = mybir.dt.float32, mybir.dt.int32

    nc = bacc.Bacc("TRN2", debug=False)
    werr = nc.dram_tensor(
        "Werr", [pool_entries * 128, WT_COLS], f32, kind="ExternalInput"
    )
    berr = nc.dram_tensor("Berr", [pool_entries, OUT], f32, kind="ExternalInput")
    wr = nc.dram_tensor("Wr", [128, WT_COLS], f32, kind="ExternalInput")
    xt = nc.dram_tensor("Xt", [128, BL * 4], f32, kind="ExternalInput")
    idx = nc.dram_tensor("idx", [128, BL], i32, kind="ExternalInput")
    loc = nc.dram_tensor("loc", [BL, 1], i32, kind="ExternalInput")
    bias16 = nc.dram_tensor("bias16", [BL, OUT], f32, kind="ExternalInput")
    eye16 = nc.dram_tensor("eye16", [BL, BL], f32, kind="ExternalInput")
    z = nc.dram_tensor("Z", [1, BL * OUT], f32, kind="ExternalOutput")

    f32r = mybir.dt.float32r

    with tile.TileContext(nc) as tc:
        with (
            tc.tile_pool(name="const", bufs=1) as cpool,
            tc.tile_pool(name="wts", bufs=6) as wpool,
            tc.tile_pool(name="prod", bufs=3) as ptpool,
            tc.tile_pool(name="ps", bufs=8, space="PSUM") as ppool,
        ):
            # idx first: the Werr gathers are gated only on this tiny load
            idx_sb = cpool.tile([128, BL], i32)
            nc.sync.dma_start(idx_sb[:], idx.ap())
            loc_sb = cpool.tile([BL, 1], i32)
            nc.sync.dma_start(loc_sb[:], loc.ap())
            wr_sb = cpool.tile([128, WT_COLS], f32)
            nc.sync.dma_start(wr_sb[:], wr.ap())
            xt_sb = cpool.tile([128, BL * 4], f32)
            nc.sync.dma_start(xt_sb[:], xt.ap())
            bias_sb = cpool.tile([BL, OUT], f32)
            nc.sync.dma_start(bias_sb[:], bias16.ap())
            eye_sb = cpool.tile([BL, BL], f32)
            nc.sync.dma_start(eye_sb[:], eye16.ap())
            zstage = cpool.tile([1, BL * OUT], f32)

            # fp32r matmul operands must be written by a rounding producer;
            # route the small stationary tensors through a DVE cast-copy.
            xtr_sb = cpool.tile([128, BL * 4], f32r)
            nc.vector.tensor_copy(xtr_sb[:], xt_sb[:])
            eyer_sb = cpool.tile([BL, BL], f32r)
            nc.vector.tensor_copy(eyer_sb[:], eye_sb[:])

            berr_sb = cpool.tile([BL, OUT], f32)
            nc.gpsimd.indirect_dma_start(
                out=berr_sb[:],
                out_offset=None,
                in_=berr.ap(),
                in_offset=bass.IndirectOffsetOnAxis(ap=loc_sb[:, :1], axis=0),
            )
            memb_sb = cpool.tile([BL, OUT], f32r)
            nc.vector.tensor_mul(memb_sb[:], berr_sb[:], bias_sb[:])

            # The last sample is gathered and multiplied in 4 j-chunks of
            # [128, 512] so the tail chain (gather -> TT -> matmuls) pipelines
            # instead of serializing behind one 1MB gather + one 2.3us TT.
            CHUNKED = {BL - 1}

            for b in range(BL):
                wt = wpool.tile([128, WT_COLS], f32, tag="wt")
                pt = ptpool.tile([128, WT_COLS], f32r, tag="pt")
                if b in CHUNKED:
                    for j in range(4):
                        nc.gpsimd.indirect_dma_start(
                            out=wt[:, j * OUT : (j + 1) * OUT],
                            out_offset=None,
                            in_=werr.ap(),
                            in_offset=bass.IndirectOffsetOnAxis(
                                ap=idx_sb[:, b : b + 1], axis=0
                            ),
                            element_offset=j * OUT,
                        )
                        nc.vector.tensor_mul(
                            pt[:, j * OUT : (j + 1) * OUT],
                            wt[:, j * OUT : (j + 1) * OUT],
                            wr_sb[:, j * OUT : (j + 1) * OUT],
                        )
                else:
                    nc.gpsimd.indirect_dma_start(
                        out=wt[:],
                        out_offset=None,
                        in_=werr.ap(),
                        in_offset=bass.IndirectOffsetOnAxis(
                            ap=idx_sb[:, b : b + 1], axis=0
                        ),
                    )
                    nc.vector.tensor_mul(pt[:], wt[:], wr_sb[:])
                ps = ppool.tile([1, OUT], f32, tag="ps")
                for j in range(4):
                    nc.tensor.matmul(
                        out=ps[:],
                        lhsT=xtr_sb[:, 4 * b + j : 4 * b + j + 1],
                        rhs=pt[:, j * OUT : (j + 1) * OUT],
                        start=(j == 0),
                        stop=False,
                    )
                nc.tensor.matmul(
                    out=ps[:],
                    lhsT=eyer_sb[:, b : b + 1],
                    rhs=memb_sb[:],
                    start=False,
                    stop=True,
                )
                nc.scalar.copy(out=zstage[0:1, b * OUT : (b + 1) * OUT], in_=ps[:])
                if b == BL // 2 - 1:
                    # first half of the output can ship while the second half
                    # is still being computed
                    nc.sync.dma_start(
                        z.ap()[:, : (BL // 2) * OUT],
                        zstage[0:1, : (BL // 2) * OUT],
                    )

            nc.sync.dma_start(
                z.ap()[:, (BL // 2) * OUT :], zstage[0:1, (BL // 2) * OUT :]
            )

    nc.compile()
    return nc


def get_nc(pool_entries=POOL):
    key = ("nc", pool_entries)
    if key not in _CACHE:
        _CACHE[key] = _build(pool_entries)
    return _CACHE[key]


def make_in_maps(X, W, bias, Werr, Berr, loc_id):
    X = np.ascontiguousarray(np.asarray(X, dtype=np.float32))
    W = np.ascontiguousarray(np.asarray(W, dtype=np.float32))
    bias = np.ascontiguousarray(np.asarray(bias, dtype=np.float32))
    Werr = np.ascontiguousarray(np.asarray(Werr, dtype=np.float32))
    Berr = np.ascontiguousarray(np.asarray(Berr, dtype=np.float32))
    loc_id = np.ascontiguousarray(np.asarray(loc_id, dtype=np.int32))

    pool_entries = Werr.shape[0]
    werr2d = Werr.reshape(pool_entries * 128, WT_COLS)
    wr = W.reshape(128, WT_COLS)
    bias16 = np.ascontiguousarray(np.broadcast_to(bias[None, :], (BL, OUT)))
    eye16 = np.eye(BL, dtype=np.float32)
    p_iota = np.arange(128, dtype=np.int32)[:, None]

    in_maps = []
    for c in range(NCORES):
        xc = X[c * BL : (c + 1) * BL]  # [BL, IN]
        locc = loc_id[c * BL : (c + 1) * BL]  # [BL]
        xt = np.ascontiguousarray(
            xc.reshape(BL, 128, 4).transpose(1, 0, 2).reshape(128, BL * 4)
        )
        idx = np.ascontiguousarray(locc[None, :] * 128 + p_iota).astype(np.int32)
        in_maps.append(
            {
                "Werr": werr2d,
                "Berr": Berr,
                "Wr": wr,
                "Xt": xt,
                "idx": idx,
                "loc": np.ascontiguousarray(locc[:, None]),
                "bias16": bias16,
                "eye16": eye16,
            }
        )
    return in_maps


def _reset_accelerator():
    import ctypes

    try:
        lib = ctypes.CDLL("/opt/axon/libaxon_pjrt.so")
        lib.axon_reset.restype = ctypes.c_int64
        lib.axon_reset()
    except Exception:
        pass


def kernel(X, W, bias, Werr, Berr, loc_id):
    from concourse.bass_utils import run_bass_kernel_spmd

    nc = get_nc()
    in_maps = make_in_maps(X, W, bias, Werr, Berr, loc_id)
    try:
        res = run_bass_kernel_spmd(nc, in_maps, core_ids=list(range(NCORES)))
    except Exception:
        # a wedged NeuronCore surfaces as an unrecoverable-device error;
        # reset the accelerator once and retry
        _reset_accelerator()
        res = run_bass_kernel_spmd(nc, in_maps, core_ids=list(range(NCORES)))
    out = np.concatenate(
        [res.results[c]["Z"].reshape(BL, OUT) for c in range(NCORES)], axis=0
    )
    return out



# revision 3
# speedup vs baseline: 1.3408x; 1.3408x over previous
"""Bass/Trainium2 kernel for DropConnect (training path, Wstd != 0).

Z[b,o] = sum_i X[b,i] * W[i,o] * Werr[loc_id[b],i,o] + bias[o] * Berr[loc_id[b],o]

Strategy (8 NeuronCores, data-parallel over batch):
  - each core handles 16 samples; loc_id is known on the host at launch, so
    the per-sample Werr/Berr rows are gathered host-side while sharding and
    shipped per-core as plain contiguous inputs (the "all-gather of the
    needed rows" sharding choice) -- no on-device indirect DMA at all
  - slabs are shipped in bf16 (tolerance is 2e-2; measured end-to-end
    rel err ~3e-3), halving HBM traffic to ~8.4 MB/core
  - the slab stream is loaded with 8 x 1MB HWDGE DMAs (hardware descriptor
    generation; the SWDGE/GpSimd path was the previous bottleneck)
  - VectorE multiplies each slab elementwise with W (all-bf16, 2x DVE mode)
  - TensorE contracts with X: per sample, 4 accumulating [128,1]x[128,512]
    matmuls into a [1,512] PSUM tile plus a 5th [16,1]x[16,512] eye-column
    matmul that adds the bias*Berr row
  - ScalarE copies each PSUM row into a [1,8192] staging tile; the output
    ships in 4 x 8KB DMAs so the store overlaps the tail of the compute
"""

import sys

sys.path.insert(0, "/opt/trn_rl_repo")

import ml_dtypes
import numpy as np

B, IN, OUT, POOL, NCORES = 128, 512, 512, 1000, 8
BL = B // NCORES  # samples per core
WT_COLS = 4 * OUT  # 2048: one macro-row = 4 input rows of W/Werr
CH = 2  # slabs per chunk DMA (1MB bf16 per chunk)
NCHUNK = BL // CH

BF16 = ml_dtypes.bfloat16

_CACHE = {}


def _build():
    import concourse.mybir as mybir
    import concourse.tile as tile
    from concourse import bacc

    f32, bf16 = mybir.dt.float32, mybir.dt.bfloat16

    nc = bacc.Bacc("TRN2", debug=False)
    wd = nc.dram_tensor("WD", [128, BL * WT_COLS], bf16, kind="ExternalInput")
    wr = nc.dram_tensor("Wr", [128, WT_COLS], bf16, kind="ExternalInput")
    xt = nc.dram_tensor("Xt", [128, BL * 4], bf16, kind="ExternalInput")
    eye = nc.dram_tensor("Eye", [BL, BL], bf16, kind="ExternalInput")
    bias16 = nc.dram_tensor("bias16", [BL, OUT], f32, kind="ExternalInput")
    berr16 = nc.dram_tensor("berr16", [BL, OUT], f32, kind="ExternalInput")
    z = nc.dram_tensor("Z", [1, BL * OUT], f32, kind="ExternalOutput")

    with tile.TileContext(nc) as tc:
        with (
            tc.tile_pool(name="const", bufs=1) as cpool,
            tc.tile_pool(name="wts", bufs=3) as wpool,
            tc.tile_pool(name="prod", bufs=2) as ptpool,
            tc.tile_pool(name="ps", bufs=8, space="PSUM") as ppool,
        ):
            # small inputs ride the Activation HWDGE ring so the slab
            # stream on the SP ring starts immediately
            wr_sb = cpool.tile([128, WT_COLS], bf16)
            nc.scalar.dma_start(wr_sb[:], wr.ap())
            xt_sb = cpool.tile([128, BL * 4], bf16)
            nc.scalar.dma_start(xt_sb[:], xt.ap())
            eye_sb = cpool.tile([BL, BL], bf16)
            nc.scalar.dma_start(eye_sb[:], eye.ap())
            bias_sb = cpool.tile([BL, OUT], f32)
            nc.scalar.dma_start(bias_sb[:], bias16.ap())
            berr_sb = cpool.tile([BL, OUT], f32)
            nc.scalar.dma_start(berr_sb[:], berr16.ap())
            memb_sb = cpool.tile([BL, OUT], bf16)
            nc.vector.tensor_mul(memb_sb[:], berr_sb[:], bias_sb[:])
            zstage = cpool.tile([1, BL * OUT], f32)

            for k in range(NCHUNK):
                wt = wpool.tile([128, CH * WT_COLS], bf16, tag="wt")
                nc.sync.dma_start(
                    wt[:], wd.ap()[:, k * CH * WT_COLS : (k + 1) * CH * WT_COLS]
                )
                pt = ptpool.tile([128, CH * WT_COLS], bf16, tag="pt")
                for c in range(CH):
                    nc.vector.tensor_mul(
                        pt[:, c * WT_COLS : (c + 1) * WT_COLS],
                        wt[:, c * WT_COLS : (c + 1) * WT_COLS],
                        wr_sb[:],
                    )
                for c in range(CH):
                    b = k * CH + c
                    ps = ppool.tile([1, OUT], f32, tag="ps")
                    for j in range(4):
                        nc.tensor.matmul(
                            out=ps[:],
                            lhsT=xt_sb[:, 4 * b + j : 4 * b + j + 1],
                            rhs=pt[
                                :, c * WT_COLS + j * OUT : c * WT_COLS + (j + 1) * OUT
                            ],
                            start=(j == 0),
                            stop=False,
                        )
                    nc.tensor.matmul(
                        out=ps[:],
                        lhsT=eye_sb[:, b : b + 1],
                        rhs=memb_sb[:],
                        start=False,
                        stop=True,
                    )
                    nc.scalar.copy(
                        out=zstage[0:1, b * OUT : (b + 1) * OUT], in_=ps[:]
                    )
                    if b % 4 == 3:
                        g = b // 4
                        nc.scalar.dma_start(
                            z.ap()[:, g * 4 * OUT : (g + 1) * 4 * OUT],
                            zstage[0:1, g * 4 * OUT : (g + 1) * 4 * OUT],
                        )

    nc.compile()
    return nc


def get_nc():
    if "nc" not in _CACHE:
        _CACHE["nc"] = _build()
    return _CACHE["nc"]


def make_in_maps(X, W, bias, Werr, Berr, loc_id):
    X = np.ascontiguousarray(np.asarray(X, dtype=np.float32))
    W = np.ascontiguousarray(np.asarray(W, dtype=np.float32))
    bias = np.ascontiguousarray(np.asarray(bias, dtype=np.float32))
    Werr = np.asarray(Werr, dtype=np.float32)
    Berr = np.asarray(Berr, dtype=np.float32)
    loc_id = np.asarray(loc_id, dtype=np.int32)

    wrb = np.ascontiguousarray(W.reshape(128, WT_COLS).astype(BF16))
    bias16 = np.ascontiguousarray(np.broadcast_to(bias[None, :], (BL, OUT)))
    eye16 = np.eye(BL, dtype=BF16)

    in_maps = []
    for c in range(NCORES):
        xc = X[c * BL : (c + 1) * BL]  # [BL, IN]
        locc = loc_id[c * BL : (c + 1) * BL]  # [BL]
        # slab b in columns [b*2048:(b+1)*2048]; partition p = in-rows 4p..4p+3
        wdc = np.ascontiguousarray(
            Werr[locc]
            .astype(BF16)
            .reshape(BL, 128, WT_COLS)
            .transpose(1, 0, 2)
            .reshape(128, BL * WT_COLS)
        )
        xtc = np.ascontiguousarray(
            xc.reshape(BL, 128, 4).transpose(1, 0, 2).reshape(128, BL * 4).astype(BF16)
        )
        in_maps.append(
            {
                "WD": wdc,
                "Wr": wrb,
                "Xt": xtc,
                "Eye": eye16,
                "bias16": bias16,
                "berr16": np.ascontiguousarray(Berr[locc]),
            }
        )
    return in_maps


def _reset_accelerator():
    import ctypes

    try:
        lib = ctypes.CDLL("/opt/axon/libaxon_pjrt.so")
        lib.axon_reset.restype = ctypes.c_int64
        lib.axon_reset()
    except Exception:
        pass


def kernel(X, W, bias, Werr, Berr, loc_id):
    from concourse.bass_utils import run_bass_kernel_spmd

    nc = get_nc()
    in_maps = make_in_maps(X, W, bias, Werr, Berr, loc_id)
    try:
        res = run_bass_kernel_spmd(nc, in_maps, core_ids=list(range(NCORES)))
    except Exception:
        # a wedged NeuronCore surfaces as an unrecoverable-device error;
        # reset the accelerator once and retry
        _reset_accelerator()
        res = run_bass_kernel_spmd(nc, in_maps, core_ids=list(range(NCORES)))
    out = np.concatenate(
        [res.results[c]["Z"].reshape(BL, OUT) for c in range(NCORES)], axis=0
    )
    return out
